# revision 16
# baseline (speedup 1.0000x reference)
"""Distributed attention kernel for Trainium2 (8 NeuronCores).

Computes, matching the reference:
    Q = x @ Wq.T + bq; K = x @ Wk.T + bk; V = x @ Wv.T + bv        [B,S,A]
    dots = Q @ K.T (per batch)                                      [B,S,S]
    attn = softmax(dots, axis=0)            # softmax over the BATCH dim
    out  = (attn @ V) @ Wp.T + bp                                   [B,S,F]

Sharding: sequence (rows of Q) is split across the 8 cores (256 rows each,
all 4 batches per core, since the batch-softmax couples batches). Each core
also computes K^T and V for its 256 sequence rows and all-gathers them
(K^T in fp16, V in bf16). dots/softmax/attn@V/proj run fully on-core.

Matmul dtypes: projections in float32r (fp32 rounded to 11 mantissa bits,
full-rate on the PE); dots in fp16; attn@V and the output projection in bf16.
All accumulation is fp32 in PSUM. exp() is taken with a constant -30 shift
(softmax is shift-invariant; keeps e^x within fp32 range for dots in [-82,90]).
"""

import numpy as np
import ml_dtypes

import concourse.bass as bass
import concourse.tile as tile
from concourse import bacc, mybir
from concourse.bass_utils import run_bass_kernel_spmd

AF = mybir.ActivationFunctionType
F32R = mybir.dt.float32r
F32 = mybir.dt.float32
F16 = mybir.dt.float16
BF16 = mybir.dt.bfloat16

B, S, F, A = 4, 2048, 1024, 1024
NCORES = 8
SQ = S // NCORES          # 256 q rows per core
NFT = F // 128            # 8 f-tiles
NAT = A // 128            # 8 a-tiles
NKT = S // 128            # 16 global k-tiles
RG = [list(range(NCORES))]


def _round_f32r(x):
    """Round fp32 to 11 mantissa bits (RNE) — the PE's fp32r input format."""
    v = np.ascontiguousarray(x, dtype=np.float32).view(np.uint32)
    r = (v >> 12) & np.uint32(1)
    v = (v + np.uint32(0x7FF) + r) & np.uint32(0xFFFFF000)
    return v.view(np.float32)


def build():
    nc = bacc.Bacc("TRN2", target_bir_lowering=False, debug=False)

    xt_ext = nc.declare_dram_parameter("xt", [B, F, SQ], F32R, isOutput=False)
    wqt_ext = nc.declare_dram_parameter("wqt", [F, A], F32R, isOutput=False)
    wkt_ext = nc.declare_dram_parameter("wkt", [F, A], F32R, isOutput=False)
    wvt_ext = nc.declare_dram_parameter("wvt", [F, A], F32R, isOutput=False)
    wpt_ext = nc.declare_dram_parameter("wpt", [A, F], BF16, isOutput=False)
    bq_ext = nc.declare_dram_parameter("bq", [NAT, 128], F32, isOutput=False)
    bk_ext = nc.declare_dram_parameter("bk", [NAT, 128], F32, isOutput=False)
    bv_ext = nc.declare_dram_parameter("bv", [1, A], F32R, isOutput=False)
    bp_ext = nc.declare_dram_parameter("bp", [1, F], BF16, isOutput=False)
    ones_r_ext = nc.declare_dram_parameter("ones_r", [1, 128], F32R, isOutput=False)
    ones_b_ext = nc.declare_dram_parameter("ones_b", [1, 128], BF16, isOutput=False)
    out_ext = nc.declare_dram_parameter("out", [B, SQ, F], F32, isOutput=True)

    with tile.TileContext(nc) as tc:
        with (
            tc.tile_pool(name="dram", bufs=1, space="DRAM") as dram,
            tc.tile_pool(name="live", bufs=1) as live,
        ):
            # ---------------- bounce + gather DRAM buffers ----------------
            kb = dram.tile([B, A, SQ], F16)              # K^T shard bounce
            kg = dram.tile([NCORES, B, A, SQ], F16, addr_space="Shared")
            vb0 = dram.tile([4, SQ, B, 128], BF16)       # V bounce, a-tiles 0..3
            vb1 = dram.tile([4, SQ, B, 128], BF16)       # V bounce, a-tiles 4..7
            vg0 = dram.tile([NCORES, 4, SQ, B, 128], BF16, addr_space="Shared")
            vg1 = dram.tile([NCORES, 4, SQ, B, 128], BF16, addr_space="Shared")

            # ---------------- whole-kernel SBUF residents ----------------
            # (bias loads are emitted after the big wk load below)
            bq_sb = live.tile([128, NAT], F32)
            bk_sb = live.tile([128, NAT], F32)
            bv_sb = live.tile([1, A], F32R)
            bp_sb = live.tile([1, F], BF16)
            ones_r = live.tile([1, 128], F32R)
            ones_b = live.tile([1, 128], BF16)
            negc = live.tile([128, 1], F32)
            nc.gpsimd.memset(negc[:], -30.0)

            qt_sb = [live.tile([128, NAT * SQ], F16, tag=f"qt{b}", name=f"qt{b}") for b in range(B)]
            W_ = [[live.tile([128, SQ], BF16, tag=f"W{b}_{kt}", name=f"W{b}_{kt}")
                   for kt in range(NKT)] for b in range(B)]

            # ============ phase A: projections (K, Q, V) ============
            psAB = tc.tile_pool(name="psAB", bufs=1, space="PSUM")
            psA = psB = psAB.__enter__()
            with (
                tc.tile_pool(name="phA", bufs=1) as phA,
                tc.tile_pool(name="kout", bufs=8) as kout,
                tc.tile_pool(name="vout", bufs=4) as vout,
            ):
                # xt + V weights load on the scalar queue, wk on sync — both
                # start at t=0 so the K projection can begin ~15us in.
                xt_sb = []
                for b in range(B):
                    t = phA.tile([128, NFT * SQ], F32R, tag=f"xt{b}", name=f"xt{b}")
                    nc.scalar.dma_start(
                        t[:].rearrange("p (ft q) -> p ft q", ft=NFT),
                        xt_ext[b].rearrange("(ft p) q -> p ft q", ft=NFT),
                    )
                    xt_sb.append(t)
                wv_sb = phA.tile([128, NFT * A], F32R, tag="wv", name="wv_sb")
                nc.scalar.dma_start(
                    wv_sb[:].rearrange("p (ft a) -> p ft a", ft=NFT),
                    wvt_ext[:].rearrange("(ft p) a -> p ft a", ft=NFT),
                )
                def load_wmat(src, tag):
                    t = phA.tile([128, NFT * NAT * 128], F32R, tag=tag, name=tag)
                    nc.sync.dma_start(
                        t[:].rearrange("p (ft at al) -> p ft at al", ft=NFT, at=NAT),
                        src[:].rearrange("(ft p) (at al) -> p ft at al", ft=NFT, al=128),
                    )
                    return t

                # ---- K^T projection (wk is the first DMA on the sync queue)
                wk_sb = load_wmat(wkt_ext, "wk")
                nc.sync.dma_start(bq_sb[:], bq_ext[:].rearrange("at p -> p at"))
                nc.sync.dma_start(bk_sb[:], bk_ext[:].rearrange("at p -> p at"))
                nc.sync.dma_start(bv_sb[:], bv_ext[:])
                nc.sync.dma_start(bp_sb[:], bp_ext[:])
                nc.sync.dma_start(ones_r[:], ones_r_ext[:])
                nc.sync.dma_start(ones_b[:], ones_b_ext[:])
                wq_sb = phA.tile([128, NFT * NAT * 128], F32R, tag="wq", name="wq")
                nc.sync.dma_start(
                    wq_sb[:].rearrange("p (ft at al) -> p ft at al", ft=NFT, at=NAT),
                    wqt_ext[:].rearrange("(ft p) (at al) -> p ft at al", ft=NFT, al=128),
                )
                for b in range(B):
                    for at in range(NAT):
                        ps = psA.tile([128, SQ], F32, tag="p256", bufs=4)
                        for ft in range(NFT):
                            nc.tensor.matmul(
                                ps[:],
                                wk_sb[:, (ft * NAT + at) * 128 : (ft * NAT + at) * 128 + 128],
                                xt_sb[b][:, ft * SQ : (ft + 1) * SQ],
                                start=(ft == 0),
                                stop=(ft == NFT - 1),
                            )
                        kt_t = kout.tile([128, SQ], F16, tag="ko")
                        nc.scalar.activation(
                            kt_t[:], ps[:], AF.Identity, bias=bk_sb[:, at : at + 1]
                        )
                        nc.scalar.dma_start(kb[b, at * 128 : (at + 1) * 128, :], kt_t[:])
                    if b == 3:
                        nc.gpsimd.collective_compute(
                            "AllGather", mybir.AluOpType.bypass, replica_groups=RG,
                            ins=[kb[:].opt()], outs=[kg[:].opt()],
                        )

                # ---- Q^T projection
                for b in range(B):
                    for at in range(NAT):
                        ps = psA.tile([128, SQ], F32, tag="p256", bufs=4)
                        for ft in range(NFT):
                            nc.tensor.matmul(
                                ps[:],
                                wq_sb[:, (ft * NAT + at) * 128 : (ft * NAT + at) * 128 + 128],
                                xt_sb[b][:, ft * SQ : (ft + 1) * SQ],
                                start=(ft == 0),
                                stop=(ft == NFT - 1),
                            )
                        nc.scalar.activation(
                            qt_sb[b][:, at * SQ : (at + 1) * SQ], ps[:],
                            AF.Identity, bias=bq_sb[:, at : at + 1],
                        )

                # ---- V projection; bounce writes on the scalar queue
                for b in range(B):
                    for st in range(2):
                        for ac in range(2):
                            ps = psB.tile([128, 512], F32, tag="p512v", bufs=2)
                            for ft in range(NFT):
                                nc.tensor.matmul(
                                    ps[:],
                                    xt_sb[b][:, ft * SQ + st * 128 : ft * SQ + st * 128 + 128],
                                    wv_sb[:, ft * A + ac * 512 : ft * A + ac * 512 + 512],
                                    start=(ft == 0),
                                    stop=False,
                                )
                            nc.tensor.matmul(
                                ps[:], ones_r[:], bv_sb[:, ac * 512 : ac * 512 + 512],
                                start=False, stop=True,
                            )
                            v_t = vout.tile([128, 512], BF16, tag="vo")
                            nc.scalar.activation(v_t[:], ps[:], AF.Copy)
                            vbx = vb0 if ac == 0 else vb1
                            nc.scalar.dma_start(
                                vbx[:, st * 128 : (st + 1) * 128, b, :]
                                .rearrange("at s al -> s at al"),
                                v_t[:].rearrange("s (at al) -> s at al", at=4),
                            )
                nc.gpsimd.collective_compute(
                    "AllGather", mybir.AluOpType.bypass, replica_groups=RG,
                    ins=[vb0[:].opt()], outs=[vg0[:].opt()],
                )
                nc.gpsimd.collective_compute(
                    "AllGather", mybir.AluOpType.bypass, replica_groups=RG,
                    ins=[vb1[:].opt()], outs=[vg1[:].opt()],
                )

            # ============ phase B: dots (fp16) + softmax over batch ============
            with (
                tc.tile_pool(name="phB", bufs=1) as phB,
                tc.tile_pool(name="ktp", bufs=4) as ktp,
                tc.tile_pool(name="scr", bufs=2) as scr,
            ):
                E = [[phB.tile([128, SQ], BF16, tag=f"E{b}_{kt}", name=f"E{b}_{kt}")
                      for kt in range(NKT)] for b in range(B)]
                for b in range(B):
                    for c in range(NCORES):
                        # both k-tiles of rank c at once: 512B contiguous runs
                        kt2 = ktp.tile([128, NAT * 2 * 128], F16, tag="kt2", name=f"kt2_{b}_{c}")
                        nc.sync.dma_start(
                            kt2[:].rearrange("p (at k) -> p at k", at=NAT),
                            kg[c, b].rearrange("(at p) k -> p at k", at=NAT),
                        )
                        for ktl in range(2):
                            kt = c * 2 + ktl
                            ps = psA.tile([128, SQ], F32, tag="p256", bufs=4)
                            for at in range(NAT):
                                nc.tensor.matmul(
                                    ps[:],
                                    kt2[:, at * 256 + ktl * 128 : at * 256 + ktl * 128 + 128],
                                    qt_sb[b][:, at * SQ : (at + 1) * SQ],
                                    start=(at == 0),
                                    stop=(at == NAT - 1),
                                )
                            nc.scalar.activation(
                                E[b][kt][:], ps[:], AF.Exp, bias=negc[:],
                            )
                            if b == B - 1:
                                d01 = scr.tile([128, SQ], F32, tag="d01")
                                nc.vector.tensor_add(d01[:], E[0][kt][:], E[1][kt][:])
                                d23 = scr.tile([128, SQ], F32, tag="d23")
                                nc.vector.tensor_add(d23[:], E[2][kt][:], E[3][kt][:])
                                dd = scr.tile([128, SQ], F32, tag="dd")
                                nc.vector.tensor_add(dd[:], d01[:], d23[:])
                                rr = scr.tile([128, SQ], F32, tag="rr")
                                nc.vector.reciprocal_approx_fast(rr[:], dd[:])
                                rb = scr.tile([128, SQ], BF16, tag="rb")
                                nc.vector.tensor_copy(rb[:], rr[:])
                                for b2 in range(B):
                                    nc.vector.tensor_mul(
                                        W_[b2][kt][:], E[b2][kt][:], rb[:]
                                    )

            psAB.__exit__(None, None, None)
            # ============ phase C: attn @ V (bf16) + output projection ============
            psCp = tc.tile_pool(name="psCp", bufs=1, space="PSUM")
            psC = psCp.__enter__()
            with (
                tc.tile_pool(name="phC", bufs=1) as phC,
                tc.tile_pool(name="vtp", bufs=6) as vtp,
                tc.tile_pool(name="oout", bufs=4) as oout,
            ):
                attT = [phC.tile([128, NAT * SQ], BF16, tag=f"attT{b}", name=f"attT{b}") for b in range(B)]
                wp_sb = phC.tile([128, NAT * F], BF16, tag="wp", name="wp_sb")
                nc.sync.dma_start(
                    wp_sb[:].rearrange("p (at f) -> p at f", at=NAT),
                    wpt_ext[:].rearrange("(at p) f -> p at f", at=NAT),
                )
                for at in range(NAT):
                    vgx = vg0 if at < 4 else vg1
                    atl = at % 4
                    ps_b = [
                        psC.tile([128, SQ], F32, tag="attnv", name=f"psatt{at}_{b}", bufs=6)
                        for b in range(B)
                    ]
                    for c in range(NCORES):
                        vt2 = vtp.tile([128, 2 * B * 128], BF16, tag="vt2", name=f"vt2_{at}_{c}")
                        eng = nc.sync if (c % 2) else nc.scalar
                        eng.dma_start(
                            vt2[:].rearrange("p (ktl b al) -> p ktl b al", ktl=2, b=B),
                            vgx[c, atl].rearrange("(ktl p) b al -> p ktl b al", ktl=2),
                        )
                        for ktl in range(2):
                            kt = c * 2 + ktl
                            for b in range(B):
                                nc.tensor.matmul(
                                    ps_b[b][:],
                                    vt2[:, (ktl * B + b) * 128 : (ktl * B + b) * 128 + 128],
                                    W_[b][kt][:],
                                    start=(kt == 0),
                                    stop=(kt == NKT - 1),
                                )
                    for b in range(B):
                        nc.scalar.activation(
                            attT[b][:, at * SQ : (at + 1) * SQ], ps_b[b][:], AF.Copy
                        )

                for b in range(B):
                    for qt in range(2):
                        for fc in range(2):
                            ps = psC.tile([128, 512], F32, tag="p512o", bufs=2)
                            for at in range(NAT):
                                nc.tensor.matmul(
                                    ps[:],
                                    attT[b][:, at * SQ + qt * 128 : at * SQ + qt * 128 + 128],
                                    wp_sb[:, at * F + fc * 512 : at * F + fc * 512 + 512],
                                    start=(at == 0),
                                    stop=False,
                                )
                            nc.tensor.matmul(
                                ps[:], ones_b[:], bp_sb[:, fc * 512 : fc * 512 + 512],
                                start=False, stop=True,
                            )
                            o_t = oout.tile([128, 512], F32, tag="ot")
                            nc.scalar.activation(o_t[:], ps[:], AF.Copy)
                            nc.sync.dma_start(
                                out_ext[b, qt * 128 : (qt + 1) * 128,
                                        fc * 512 : (fc + 1) * 512],
                                o_t[:],
                            )
            psCp.__exit__(None, None, None)

    nc.finalize()
    return nc


_NC_CACHE = None


def _get_nc():
    global _NC_CACHE
    if _NC_CACHE is None:
        _NC_CACHE = build()
    return _NC_CACHE


def kernel(x, Wq, bq, Wk, bk, Wv, bv, Wp, bp, _trace=False):
    x = np.asarray(x, dtype=np.float32)
    Wq = np.asarray(Wq, dtype=np.float32)
    Wk = np.asarray(Wk, dtype=np.float32)
    Wv = np.asarray(Wv, dtype=np.float32)
    Wp = np.asarray(Wp, dtype=np.float32)
    bq = np.asarray(bq, dtype=np.float32)
    bk = np.asarray(bk, dtype=np.float32)
    bv = np.asarray(bv, dtype=np.float32)
    bp = np.asarray(bp, dtype=np.float32)

    wqt = _round_f32r(np.ascontiguousarray(Wq.T))
    wkt = _round_f32r(np.ascontiguousarray(Wk.T))
    wvt = _round_f32r(np.ascontiguousarray(Wv.T))
    wpt = np.ascontiguousarray(Wp.T).astype(ml_dtypes.bfloat16)
    bq_p = np.ascontiguousarray(bq.reshape(NAT, 128))
    bk_p = np.ascontiguousarray(bk.reshape(NAT, 128))
    bv_p = _round_f32r(bv.reshape(1, A))
    bp_p = bp.reshape(1, F).astype(ml_dtypes.bfloat16)
    ones_r = np.ones((1, 128), np.float32)
    ones_b = np.ones((1, 128), ml_dtypes.bfloat16)

    in_maps = []
    for c in range(NCORES):
        xt_c = _round_f32r(
            np.ascontiguousarray(x[:, c * SQ : (c + 1) * SQ, :].transpose(0, 2, 1))
        )
        in_maps.append({
            "xt": xt_c, "wqt": wqt, "wkt": wkt, "wvt": wvt, "wpt": wpt,
            "bq": bq_p, "bk": bk_p, "bv": bv_p, "bp": bp_p,
            "ones_r": ones_r, "ones_b": ones_b,
        })

    nc = _get_nc()
    res = run_bass_kernel_spmd(
        nc, in_maps, core_ids=list(range(NCORES)), trace=_trace
    )
    out = np.concatenate([res.results[c]["out"] for c in range(NCORES)], axis=1)
    if _trace:
        kernel.last_results = res
    return out
